# revision 17
# baseline (speedup 1.0000x reference)
"""Trainium2 Bass kernel for 2-layer GAT (nn_GAT_72619307041134).

Strategy (dst-stationary edge layout, 8 cores SPMD):
- Nodes are sharded into 8 contiguous ranges of 6250.  Within each shard,
  nodes are re-ranked by (A-indegree desc, B-indegree desc); window =
  rank//128, partition = rank%128.  All device tensors (tables, xT, output)
  live in this permuted order; the host un-permutes the final output.
- Per layer, a per-node gather table lives in DRAM (bf16 rows, attention
  logits bit-packed as f32-in-bf16-slots):
    table1[n] = [al_src (8 f32 = 16 slots) | h1 (128 bf16) | pad]   512B rows
    table2[n] = [al2_src (1 f32 = 2 slots) | h2 (64 bf16) | pad]    256B rows
  Dst-side logits stay on-chip in per-window SBUF blocks (bf16).
- Edges of window w form kA[w] A-pure + kB[w] B-pure 128-edge chunks
  (A = src core < 5, so the int16 gather index fits either table view);
  chunk j, partition p holds the j-th A(B)-edge of the node at rank p, so
  the edge sits on ITS dst node's partition: the segment-sum is an
  identity-weight bf16 matmul accumulating in PSUM, and the dst logit is a
  plain per-partition broadcast.  Pad slots are zeroed via a host-built
  bf16 mask applied to ee = exp(leaky_relu(al_s + al_d)).
  Gather calls stripe round-robin over all 4 SWDGE queues (disjoint Q7
  core pairs -> parallel descriptor generation).
  Window epilogue divides by the softmax denominator (alpha =
  ee/(sum ee + 1e-16); the segment-max shift is skipped because logits are
  bounded by ~4).
- Between layers one AllGather shares each core's table shard.
- Host preprocessing only sorts/pads/permutes integer indices.
"""

from contextlib import ExitStack

import numpy as np

# ---------------------------------------------------------------------------
# config
# ---------------------------------------------------------------------------


class Cfg:
    def __init__(self, N=50000, E=800000, NCORES=8, WIN=128, CHUNK=128,
                 SPLIT=31250):
        self.N = N
        self.E = E
        self.NCORES = NCORES
        self.NSH = N // NCORES
        self.WIN = WIN
        self.NWIN = (self.NSH + WIN - 1) // WIN
        self.CHUNK = CHUNK
        self.SPLIT = SPLIT        # src < SPLIT -> table A view, else B view
        self.HEADS = 8
        self.HID = 16
        self.OUT_C = 64
        self.R1 = 256             # bf16 elems/row: [al_s 16 slots | h1 128 | pad]
        self.H1OFF = 16
        self.R2 = 128             # bf16 elems/row: [al2_s 2 slots | h2 64 | pad]
        self.H2OFF = 2
        self.MSG1 = 128 + 8
        self.MSG2 = 64 + 1
        self.NQUEUES = 4


FULL = Cfg()

# ---------------------------------------------------------------------------
# host-side edge preprocessing (indices only)
# ---------------------------------------------------------------------------


def prep_edges(edge_index: np.ndarray, cfg: Cfg):
    """Builds the dst-stationary SPMD chunk schedule and per-core arrays.

    meta: sched = [(kA, kB, chunk_base)] per window (SPMD-common), TC, KMAX.
    per_core[c]: int16 'src_idx' [128, TC*8] (wrapped: stream position
    i = chunk*128 + partition at [i%16, i//16], tiled to 128 rows),
    int16 'emask' [128, TC] (bf16 bits: 0x3F80 where the slot holds a real
    edge, 0 for pads).  perm[c]: original global node id at rank i.
    """
    N, NSH, WIN, NWIN = cfg.N, cfg.NSH, cfg.WIN, cfg.NWIN
    SPLIT = cfg.SPLIT
    loops = np.arange(N, dtype=np.int64)
    src = np.concatenate([edge_index[0].astype(np.int64), loops])
    dst = np.concatenate([edge_index[1].astype(np.int64), loops])
    isA = src < SPLIT            # src core < 5: stable under within-core perm
    core_of = dst // NSH

    # pass 1: per-core node ranking by (A-indeg desc, B-indeg desc)
    rank = np.empty(N, np.int64)      # global node -> rank within its core
    perm = []                         # core -> rank -> original global id
    dA_by_c = []
    dB_by_c = []
    for c in range(cfg.NCORES):
        m = core_of == c
        dl = dst[m] - c * NSH
        al = isA[m]
        dA = np.bincount(dl[al], minlength=NSH)
        dB = np.bincount(dl[~al], minlength=NSH)
        order = np.lexsort((-dB, -dA))        # rank -> local node
        r = np.empty(NSH, np.int64)
        r[order] = np.arange(NSH)
        rank[c * NSH:(c + 1) * NSH] = r
        perm.append(c * NSH + order)
        dA_by_c.append(dA[order])             # per-rank degrees
        dB_by_c.append(dB[order])

    new_id = (np.arange(N) // NSH) * NSH + rank   # old global -> new global

    # SPMD-common chunk counts: max over cores per window
    kA = np.zeros(NWIN, np.int64)
    kB = np.zeros(NWIN, np.int64)
    for c in range(cfg.NCORES):
        a2 = dA_by_c[c].copy()
        b2 = dB_by_c[c].copy()
        if NSH % WIN:
            pad = NWIN * WIN - NSH
            a2 = np.concatenate([a2, np.zeros(pad, np.int64)])
            b2 = np.concatenate([b2, np.zeros(pad, np.int64)])
        kA = np.maximum(kA, a2.reshape(NWIN, WIN).max(1))
        kB = np.maximum(kB, b2.reshape(NWIN, WIN).max(1))
    base = np.concatenate([[0], np.cumsum(kA + kB)])
    TC = int(base[-1])
    KMAX = int((kA + kB).max())
    sched = [(int(kA[w]), int(kB[w]), int(base[w])) for w in range(NWIN)]

    # pass 2: per-core edge placement
    per_core = []
    for c in range(cfg.NCORES):
        m = core_of == c
        se = src[m]
        rk = rank[dst[m]]
        cls = (~isA[m]).astype(np.int64)      # 0 = A, 1 = B
        key = rk * 2 + cls
        o = np.argsort(key, kind="stable")
        ks = key[o]
        first = np.r_[0, np.flatnonzero(np.diff(ks)) + 1]
        runid = np.zeros(len(ks), np.int64)
        runid[first[1:]] = 1
        runid = np.cumsum(runid)
        j = np.arange(len(ks)) - first[runid]
        rko = rk[o]
        w = rko // WIN
        p = rko % WIN
        cl = ks & 1
        chunk = base[w] + np.where(cl == 0, j, kA[w] + j)
        assert (j < np.where(cl == 0, kA[w], kB[w])).all()
        pos = chunk * 128 + p
        sidx = np.zeros(TC * 128, np.int64)
        emask = np.zeros(TC * 128, np.uint16)
        sn = new_id[se[o]]
        sidx[pos] = np.where(cl == 0, sn, sn - SPLIT)
        emask[pos] = 0x3F80                  # bf16 1.0

        def wrap(st):
            n = st.shape[0]
            out = np.zeros((16, n // 16), np.int16)
            idx = np.arange(n)
            out[idx % 16, idx // 16] = st
            return np.tile(out, (8, 1))

        per_core.append(dict(
            src_idx=wrap(sidx.astype(np.int16)),
            emask=np.ascontiguousarray(
                emask.reshape(TC, 128).T).view(np.int16),
        ))
    meta = dict(sched=tuple(sched), TC=TC, KMAX=KMAX)
    return meta, per_core, perm, new_id


def host_tensors(inputs, perm, cfg: Cfg):
    x = np.ascontiguousarray(inputs["x"], np.float32)
    W1 = np.ascontiguousarray(inputs["W1"], np.float32)
    a1s = np.asarray(inputs["a1_src"], np.float32)
    a1d = np.asarray(inputs["a1_dst"], np.float32)
    W2 = np.ascontiguousarray(inputs["W2"], np.float32)
    a2s = np.asarray(inputs["a2_src"], np.float32).reshape(1, -1)
    a2d = np.asarray(inputs["a2_dst"], np.float32).reshape(1, -1)
    b1 = np.asarray(inputs["b1"], np.float32)
    b2 = np.asarray(inputs["b2"], np.float32)
    H, HID = cfg.HEADS, cfg.HID
    A1 = np.zeros((H * HID, 2 * H), np.float32)
    for h in range(H):
        A1[h * HID:(h + 1) * HID, h] = a1s[h]
        A1[h * HID:(h + 1) * HID, H + h] = a1d[h]
    RHS2 = np.concatenate([W2 @ a2s.T, W2 @ a2d.T, W2], 1).astype(np.float32)
    shared = dict(W1=W1, A1=A1, RHS2=RHS2,
                  b1row=b1.reshape(1, -1),
                  b2row=np.concatenate([np.zeros(2, np.float32), b2]).reshape(1, -1))
    xT = [np.ascontiguousarray(x[perm[c]].T) for c in range(cfg.NCORES)]
    has_bias = bool(np.any(b1) or np.any(b2))
    return shared, xT, has_bias


# ---------------------------------------------------------------------------
# device kernel emission
# ---------------------------------------------------------------------------


def _ap(base, free_dims, extra_off=0):
    """Replace the free dims of a [P, ...] AP (keep partition dim)."""
    import concourse.bass as bass

    return bass.AP(base.tensor, base.offset + extra_off,
                   [list(base.ap[0])] + [list(d) for d in free_dims])


def emit_gat(tc, out_ap, ins, meta, cfg: Cfg, has_bias=False):
    import concourse.bass as bass  # noqa: F401
    from concourse import mybir

    nc = tc.nc
    f32 = mybir.dt.float32
    bf16 = mybir.dt.bfloat16
    i16 = mybir.dt.int16
    i32 = mybir.dt.int32
    AF = mybir.ActivationFunctionType
    OP = mybir.AluOpType
    N, NSH, WIN, NWIN = cfg.N, cfg.NSH, cfg.WIN, cfg.NWIN
    TC, KMAX = meta["TC"], meta["KMAX"]
    sched = meta["sched"]
    NQ = cfg.NQUEUES
    R1, R2 = cfg.R1, cfg.R2
    M1, M2 = cfg.MSG1, cfg.MSG2
    NIW = TC * cfg.CHUNK // 16

    ctx = ExitStack()
    with ctx:
        dram = ctx.enter_context(tc.tile_pool(name="dram", bufs=1, space="DRAM"))
        consts = ctx.enter_context(tc.tile_pool(name="consts", bufs=1))

        t1shard = dram.tile([NSH, R1], bf16)
        t1full = dram.tile([N, R1], bf16, addr_space="Shared")
        t2shard = dram.tile([NSH, R2], bf16)
        t2full = dram.tile([N, R2], bf16, addr_space="Shared")

        # ------- constants into SBUF -------
        W1_sb = consts.tile([128, 128], f32)
        A1_sb = consts.tile([128, 16], f32)
        RHS2_sb = consts.tile([128, 66], f32)
        nc.sync.dma_start(W1_sb[:], ins["W1"][:])
        nc.sync.dma_start(A1_sb[:], ins["A1"][:])
        nc.sync.dma_start(RHS2_sb[:], ins["RHS2"][:])
        W1_bf = consts.tile([128, 128], bf16)
        A1_bf = consts.tile([128, 16], bf16)
        RHS2_bf = consts.tile([128, 66], bf16)
        nc.vector.tensor_copy(W1_bf[:], W1_sb[:])
        nc.vector.tensor_copy(A1_bf[:], A1_sb[:])
        nc.vector.tensor_copy(RHS2_bf[:], RHS2_sb[:])
        src_sb = consts.tile([128, NIW], i16)
        mask_sb = consts.tile([128, TC], i16)
        nc.sync.dma_start(src_sb[:], ins["src_idx"][:])
        nc.sync.dma_start(mask_sb[:], ins["emask"][:])
        mask_bf = mask_sb[:].bitcast(bf16)
        # per-window dst-side attention logits, kept on-chip (bf16)
        ald_all = consts.tile([128, NWIN * 8], bf16)
        ald2_all = consts.tile([128, NWIN], bf16)
        nc.vector.memset(ald_all[:], 0.0)
        nc.vector.memset(ald2_all[:], 0.0)
        if has_bias:
            b1_sb = consts.tile([1, 128], f32)
            b2_sb = consts.tile([1, 66], f32)
            ones_sb = consts.tile([1, 128], f32)
            nc.sync.dma_start(b1_sb[:], ins["b1row"][:])
            nc.sync.dma_start(b2_sb[:], ins["b2row"][:])
            nc.gpsimd.memset(ones_sb[:], 1.0)

        # identity + iota
        iota_i = consts.tile([128, 128], i32)
        icol = consts.tile([128, 1], i32)
        id_sb = consts.tile([128, 128], f32)
        id_bf = consts.tile([128, 128], bf16)
        iota_f = consts.tile([128, 128], f32)
        icol_f = consts.tile([128, 1], f32)
        nc.gpsimd.iota(iota_i[:], pattern=[[1, 128]], base=0, channel_multiplier=0)
        nc.gpsimd.iota(icol[:], pattern=[[1, 1]], base=0, channel_multiplier=1)
        nc.vector.tensor_copy(iota_f[:], iota_i[:])
        nc.vector.tensor_copy(icol_f[:], icol[:])
        nc.vector.tensor_scalar(id_sb[:], iota_f[:], icol_f[:], None, OP.is_equal)
        nc.vector.tensor_copy(id_bf[:], id_sb[:])

        # ------- stage A: h1, al, table1, ald -------
        with tc.tile_pool(name="stageA", bufs=1) as sa, \
             tc.tile_pool(name="stageApsum", bufs=2, space="PSUM") as sap, \
             tc.tile_pool(name="rows", bufs=3) as rows:
            xT_sb = sa.tile([128, NSH], f32)
            nc.sync.dma_start(xT_sb[:], ins["xT"][:])
            xT_bf = sa.tile([128, NSH], bf16)
            nc.vector.tensor_copy(xT_bf[:], xT_sb[:])
            h1T_bf = sa.tile([128, NSH], bf16)
            al_sb = sa.tile([16, NSH], f32)
            nt = (NSH + 511) // 512
            for jt in range(nt):
                w0 = jt * 512
                w1 = min(NSH, w0 + 512)
                ph = sap.tile([128, 512], f32, tag="ph")
                nc.tensor.matmul(ph[:, : w1 - w0], W1_bf[:], xT_bf[:, w0:w1],
                                 start=True, stop=True)
                nc.vector.tensor_copy(h1T_bf[:, w0:w1], ph[:, : w1 - w0])
            for jt in range(nt):
                w0 = jt * 512
                w1 = min(NSH, w0 + 512)
                pa = sap.tile([16, 512], f32, tag="pa")
                nc.tensor.matmul(pa[:, : w1 - w0], A1_bf[:], h1T_bf[:, w0:w1],
                                 start=True, stop=True)
                nc.vector.tensor_copy(al_sb[:, w0:w1], pa[:, : w1 - w0])

            for w in range(NWIN):
                w0 = w * WIN
                wn = min(WIN, NSH - w0)
                hp = sap.tile([128, 128], f32, tag="hp")
                if has_bias:
                    nc.tensor.matmul(hp[:wn, :], xT_bf[:, w0:w0 + wn], W1_bf[:],
                                     start=True, stop=False)
                    nc.tensor.matmul(hp[:wn, :], ones_sb[0:1, :wn], b1_sb[:],
                                     start=False, stop=True)
                else:
                    nc.tensor.matmul(hp[:wn, :], xT_bf[:, w0:w0 + wn], W1_bf[:],
                                     start=True, stop=True)
                at = sap.tile([128, 16], f32, tag="at")
                nc.tensor.transpose(at[:wn, :], al_sb[:, w0:w0 + wn], id_sb[:16, :16])
                rowt = rows.tile([128, R1], bf16, tag="rowt")
                nc.vector.tensor_copy(rowt[:wn, 0:16].bitcast(f32), at[:wn, 0:8])
                nc.scalar.copy(rowt[:wn, 16:144], hp[:wn, :])
                nc.vector.memset(rowt[:wn, 144:R1], 0.0)
                nc.sync.dma_start(t1shard.opt()[w0:w0 + wn, :], rowt[:wn, :])
                nc.vector.tensor_copy(ald_all[:wn, w * 8:(w + 1) * 8],
                                      at[:wn, 8:16])

        from concourse import library_config

        nc.gpsimd.load_library(library_config.mlp)

        nc.gpsimd.collective_compute(
            "AllGather", mybir.AluOpType.bypass,
            replica_groups=[list(range(cfg.NCORES))],
            ins=[t1shard.opt()], outs=[t1full.opt()],
        )

        # ------- edge layers -------
        qctr = [0]

        def edge_layer(table_full, row, hoff, nh, chper, msgc, ald_win, epilogue):
            CAP = 8
            with tc.tile_pool(name="edges", bufs=24) as epool, \
                 tc.tile_pool(name="emsg", bufs=8) as mp, \
                 tc.tile_pool(name="epsum", bufs=3, space="PSUM") as pp, \
                 tc.tile_pool(name="esmall", bufs=8) as spool:
                for w in range(NWIN):
                    kA, kB, c0 = sched[w]
                    # A/B-pure pieces of at most CAP chunks
                    pieces = []
                    for s0_, n_, sh in ((0, kA, 0), (kA, kB, 1)):
                        off = 0
                        while off < n_:
                            m = min(CAP, n_ - off)
                            pieces.append((s0_ + off, m, sh))
                            off += m
                    pw = pp.tile([128, msgc], f32, tag="pw", name="pw")
                    for pi, (ks, nch, shift) in enumerate(pieces):
                        gb = epool.tile([128, CAP * row], bf16, tag="gb")
                        gb3 = gb[:].rearrange("p (k e) -> p k e", k=CAP)
                        tbl = (table_full.opt() if shift == 0
                               else table_full.opt()[cfg.SPLIT:N, :])
                        nc.gpsimd.dma_gather(
                            gb3[:, 0:nch, :], tbl,
                            src_sb[:, (c0 + ks) * 8:(c0 + ks + nch) * 8],
                            num_idxs=nch * 128, num_idxs_reg=nch * 128,
                            elem_size=row, single_packet=False,
                            queue_num=qctr[0] % NQ,
                        )
                        qctr[0] += 1
                        # logits: al_s (gathered, f32-packed) + al_d (bcast)
                        lg = spool.tile([128, CAP * nh], f32, tag="lg")
                        nc.vector.tensor_tensor(
                            _ap(lg[:], [[nh, nch], [1, nh]]),
                            _ap(gb[:].bitcast(f32), [[row // 2, nch], [1, nh]]),
                            _ap(ald_win(w), [[0, nch], [1, nh]]),
                            OP.add,
                        )
                        ee = spool.tile([128, CAP * nh], f32, tag="ee")
                        nc.vector.scalar_tensor_tensor(
                            _ap(ee[:], [[nh, nch], [1, nh]]),
                            _ap(lg[:], [[nh, nch], [1, nh]]), 0.2,
                            _ap(lg[:], [[nh, nch], [1, nh]]), OP.mult, OP.max)
                        nc.scalar.activation(
                            _ap(ee[:], [[nh, nch], [1, nh]]),
                            _ap(ee[:], [[nh, nch], [1, nh]]), AF.Exp)
                        # zero the pad slots
                        eem = spool.tile([128, CAP * nh], f32, tag="eem")
                        nc.vector.tensor_tensor(
                            _ap(eem[:], [[nh, nch], [1, nh]]),
                            _ap(ee[:], [[nh, nch], [1, nh]]),
                            _ap(mask_bf, [[1, nch], [0, nh]], c0 + ks),
                            OP.mult,
                        )
                        msg = mp.tile([128, CAP * msgc], bf16, tag="msg")
                        nc.vector.tensor_tensor(
                            _ap(msg[:], [[msgc, nch], [chper, nh], [1, chper]]),
                            _ap(gb[:], [[row, nch], [chper, nh], [1, chper]], hoff),
                            _ap(eem[:], [[nh, nch], [1, nh], [0, chper]]),
                            OP.mult,
                        )
                        nc.scalar.copy(
                            _ap(msg[:], [[msgc, nch], [1, nh]], msgc - nh),
                            _ap(eem[:], [[nh, nch], [1, nh]]),
                        )
                        # identity-weight segment sum into the window's PSUM
                        for k in range(nch):
                            nc.tensor.matmul(
                                pw[:], id_bf[:],
                                msg[:, k * msgc:(k + 1) * msgc],
                                start=(pi == 0 and k == 0),
                                stop=(pi == len(pieces) - 1 and k == nch - 1),
                            )
                    epilogue(w, pw)

        # ---- L1 ----
        with tc.tile_pool(name="epi1", bufs=2) as hq, \
             tc.tile_pool(name="epi1p", bufs=1, space="PSUM") as hpp:
            def epi1(w, pw):
                w0 = w * WIN
                wn = min(WIN, NSH - w0)
                dn = hq.tile([128, 8], f32, tag="dn")
                nc.vector.tensor_scalar(dn[:], pw[:, 128:136], 1e-16, None, OP.add)
                rcp = hq.tile([128, 8], f32, tag="rcp")
                nc.vector.reciprocal(rcp[:], dn[:])
                hb = hq.tile([128, 128], bf16, tag="hb")
                nc.vector.tensor_tensor(
                    _ap(hb[:], [[16, 8], [1, 16]]),
                    _ap(pw[:], [[16, 8], [1, 16]]),
                    _ap(rcp[:], [[1, 8], [0, 16]]),
                    OP.mult,
                )
                nc.scalar.activation(hb[:], hb[:], AF.Relu)
                tp = hpp.tile([128, 128], bf16, tag="tp")
                nc.tensor.transpose(tp[:], hb[:], id_bf[:])
                tH = hq.tile([128, 128], bf16, tag="tH")
                nc.vector.tensor_copy(tH[:], tp[:])
                p2 = hpp.tile([128, 66], f32, tag="p2")
                if has_bias:
                    nc.tensor.matmul(p2[:], tH[:], RHS2_bf[:], start=True, stop=False)
                    nc.tensor.matmul(p2[:], ones_sb[0:1, :128], b2_sb[:],
                                     start=False, stop=True)
                else:
                    nc.tensor.matmul(p2[:], tH[:], RHS2_bf[:], start=True, stop=True)
                t2b = hq.tile([128, R2], bf16, tag="t2b")
                nc.vector.tensor_copy(t2b[:wn, 0:2].bitcast(f32), p2[:wn, 0:1])
                nc.scalar.copy(t2b[:wn, 2:66], p2[:wn, 2:66])
                nc.vector.memset(t2b[:wn, 66:R2], 0.0)
                nc.sync.dma_start(t2shard.opt()[w0:w0 + wn, :], t2b[:wn, :])
                nc.vector.tensor_copy(ald2_all[:wn, w:w + 1], p2[:wn, 1:2])

            edge_layer(t1full, R1, cfg.H1OFF, 8, 16, M1,
                       lambda w: ald_all[:, w * 8:(w + 1) * 8], epi1)

        nc.gpsimd.collective_compute(
            "AllGather", mybir.AluOpType.bypass,
            replica_groups=[list(range(cfg.NCORES))],
            ins=[t2shard.opt()], outs=[t2full.opt()],
        )

        # ---- L2 ----
        with tc.tile_pool(name="epi2", bufs=2) as oq:
            def epi2(w, pw):
                w0 = w * WIN
                wn = min(WIN, NSH - w0)
                dn2 = oq.tile([128, 1], f32, tag="dn2")
                nc.vector.tensor_scalar(dn2[:], pw[:, 64:65], 1e-16, None, OP.add)
                rcp2 = oq.tile([128, 1], f32, tag="rcp2")
                nc.vector.reciprocal(rcp2[:], dn2[:])
                ob = oq.tile([128, 64], f32, tag="ob")
                nc.vector.tensor_scalar(ob[:], pw[:, 0:64], rcp2[:], None, OP.mult)
                nc.sync.dma_start(out_ap[w0:w0 + wn, :], ob[:wn, :])

            edge_layer(t2full, R2, cfg.H2OFF, 1, 64, M2,
                       lambda w: ald2_all[:, w:w + 1], epi2)


# ---------------------------------------------------------------------------
# SPMD build + run
# ---------------------------------------------------------------------------

_CACHE = {}


def _build(meta, cfg: Cfg, has_bias: bool):
    key = (meta["sched"], cfg.N, cfg.NCORES, has_bias)
    if key in _CACHE:
        return _CACHE[key]
    import concourse.tile as tile
    from concourse import bacc, mybir

    f32 = mybir.dt.float32
    i16 = mybir.dt.int16
    TC = meta["TC"]
    NIW = TC * cfg.CHUNK // 16
    nc = bacc.Bacc("TRN2", target_bir_lowering=False, debug=False,
                   num_devices=cfg.NCORES,
                   num_swdge_queues=cfg.NQUEUES)
    ins = {}

    def di(name, shape, dt=f32):
        ins[name] = nc.dram_tensor(name, shape, dt, kind="ExternalInput").ap()

    di("xT", [128, cfg.NSH])
    di("W1", [128, 128])
    di("A1", [128, 16])
    di("RHS2", [128, 66])
    di("src_idx", [128, NIW], i16)
    di("emask", [128, TC], i16)
    if has_bias:
        di("b1row", [1, 128])
        di("b2row", [1, 66])
    out = nc.dram_tensor("out", [cfg.NSH, cfg.OUT_C], f32, kind="ExternalOutput").ap()

    with tile.TileContext(nc) as tc:
        emit_gat(tc, out, ins, meta, cfg, has_bias)
    nc.compile()
    _CACHE[key] = nc
    return nc


def kernel(**inputs) -> np.ndarray:
    out, _ = _run(inputs)
    return out


def _run(inputs, **run_kwargs):
    cfg = FULL
    inputs = {k: np.asarray(v) for k, v in inputs.items()}
    edge_index = inputs["edge_index"].astype(np.int64)
    meta, per_core, perm, new_id = prep_edges(edge_index, cfg)
    shared, xT, has_bias = host_tensors(inputs, perm, cfg)
    nc = _build(meta, cfg, has_bias)

    from concourse.bass_utils import run_bass_kernel_spmd

    in_maps = []
    for c in range(cfg.NCORES):
        m = {k: shared[k] for k in ("W1", "A1", "RHS2")}
        if has_bias:
            m["b1row"] = shared["b1row"]
            m["b2row"] = shared["b2row"]
        m["xT"] = xT[c]
        m.update(per_core[c])
        in_maps.append(m)
    res = run_bass_kernel_spmd(nc, in_maps, core_ids=list(range(cfg.NCORES)),
                               **run_kwargs)
    out = np.concatenate([res.results[c]["out"] for c in range(cfg.NCORES)], 0)
    return out[new_id].astype(np.float32), res
